# revision 1
# baseline (speedup 1.0000x reference)
"""Multi-head attention forward on 8 Trainium2 NeuronCores.

Problem: x[4,2048,1024], W_attn[3072,1024], W_proj[1024,1024], b_proj[1024]
  qkv = x @ W_attn.T ; per-head softmax(q k^T / sqrt(64)) @ v ; out = y @ W_proj.T + b

Sharding: core = (batch b, head-group hg), b = core//2, hg = core%2.
Each core computes its batch's attention output for its 8 heads plus the
partial output projection over its 512 y-channels; the host sums the two
partials per batch and adds the bias.

On-core layout (all fp32):
  - The contraction of every matmul must sit on the SBUF partition dim, so the
    host ships x and the weights pre-transposed: xT[c,t], wqkT[c,o], wvT[c,o],
    wpT[c_local,o].
  - q,k are produced transposed (qkT[o,t]); attention scores are computed as
    s^T[k,q] = (kT)^T-style matmuls with K=64, two heads packed into the
    128-row PE array (rows 0:64 / 64:128).
  - softmax runs without max-subtraction (inputs are ~N(0,1) after scaling so
    exp never overflows); exp is fused with the 1/8 scale on the scalar engine.
  - v carries an extra all-ones column per head, so the p@v matmul's 65th
    output row is the softmax denominator for free.
  - y^T is normalized via a DRAM round-trip of the 16K denominators
    (reciprocal on DVE, then partition-broadcast DMA loads) and fed straight
    into the output projection as the stationary operand.
"""

import sys

import numpy as np

if "/opt/trn_rl_repo" not in sys.path:
    sys.path.insert(0, "/opt/trn_rl_repo")

B, T, C, H, D = 4, 2048, 1024, 16, 64
HPG = H // 2          # heads per core group = 8
CL = HPG * D          # local y-channels = 512
KC = C // 128         # 8 contraction tiles over c
NT = T // 128         # 16 tiles over t
NCORES = 8

_cache = {}


def _build():
    import concourse.bacc as bacc
    import concourse.bass as bass
    import concourse.mybir as mybir
    import concourse.tile as tile
    from concourse.bass import ds, ts

    f32 = mybir.dt.float32
    EXP = mybir.ActivationFunctionType.Exp

    nc = bacc.Bacc("TRN2", target_bir_lowering=False, debug=False,
                   enable_asserts=False)

    xT = nc.dram_tensor("xT", [C, T], f32, kind="ExternalInput").ap()
    wqkT = nc.dram_tensor("wqkT", [C, 2 * CL], f32, kind="ExternalInput").ap()
    wvT = nc.dram_tensor("wvT", [C, CL], f32, kind="ExternalInput").ap()
    wpT = nc.dram_tensor("wpT", [CL, C], f32, kind="ExternalInput").ap()
    out = nc.dram_tensor("out", [T, C], f32, kind="ExternalOutput").ap()
    den_dram = nc.dram_tensor("den_scr", [HPG, T], f32, kind="Internal").ap()
    rec_dram = nc.dram_tensor("rec_scr", [HPG, T], f32, kind="Internal").ap()

    with tile.TileContext(nc) as tc:
        with tc.tile_pool(name="pers", bufs=1) as pers:
            # persistent: q/k transposed [o,t] (tiles 0-3 q, 4-7 k; head pair
            # 2m/2m+1 in rows 0:64/64:128) and v in [t, head, d+ones] layout
            qkt = [pers.tile([128, T], f32, name=f"qkt{m}", tag=f"qkt{m}")
                   for m in range(8)]
            vbuf = [pers.tile([128, HPG, D + 1], f32, name=f"vb{t}",
                              tag=f"vb{t}") for t in range(NT)]

            # ---------- phase 1: qkv projection ----------
            with tc.tile_pool(name="p1w", bufs=1) as p1w, \
                 tc.tile_pool(name="p1x", bufs=2) as p1x, \
                 tc.tile_pool(name="p1qk", bufs=2, space="PSUM") as p1qk, \
                 tc.tile_pool(name="p1v", bufs=2, space="PSUM") as p1v:
                wqk_sb = [p1w.tile([128, 2 * CL], f32, name=f"wqk{k}",
                                   tag=f"wqk{k}") for k in range(KC)]
                wv_sb = [p1w.tile([128, CL], f32, name=f"wv{k}",
                                  tag=f"wv{k}") for k in range(KC)]
                for k in range(KC):
                    nc.sync.dma_start(wqk_sb[k], wqkT[ts(k, 128), :])
                    nc.sync.dma_start(wv_sb[k], wvT[ts(k, 128), :])

                for half in range(2):
                    xq = {}
                    for k in range(KC):
                        for nq in range(2):
                            qq = 2 * half + nq
                            xt = p1x.tile([128, 512], f32, name=f"xq{k}_{qq}",
                                          tag=f"xq{k}")
                            nc.sync.dma_start(
                                xt, xT[ts(k, 128), ts(qq, 512)])
                            xq[(k, qq)] = xt
                    # qk projection: qkt[m][o, t_half] += wqk^T x
                    for m in range(8):
                        qps = p1qk.tile([128, 1024], f32, name="qps",
                                        tag="qps")
                        for k in range(KC):
                            for nq in range(2):
                                nc.tensor.matmul(
                                    qps[:, ts(nq, 512)],
                                    wqk_sb[k][:, ts(m, 128)],
                                    xq[(k, 2 * half + nq)],
                                    start=(k == 0), stop=(k == KC - 1))
                        nc.scalar.copy(qkt[m][:, ds(half * 1024, 1024)], qps)
                    # v projection into [t, head, d] with ones column
                    for tl in range(8):
                        tt = half * 8 + tl
                        vps = p1v.tile([128, 512], f32, name="vps", tag="vps")
                        for k in range(KC):
                            nc.tensor.matmul(
                                vps,
                                xq[(k, 2 * half + tl // 4)][:, ds((tl % 4) * 128, 128)],
                                wv_sb[k],
                                start=(k == 0), stop=(k == KC - 1))
                        nc.gpsimd.memset(vbuf[tt][:, :, D:D + 1], 1.0)
                        nc.vector.tensor_copy(
                            vbuf[tt][:, :, 0:D],
                            vps.rearrange("p (h d) -> p h d", d=D))

            # ---------- phase 2: attention ----------
            with tc.tile_pool(name="yout", bufs=1) as youtp:
                youtT = [youtp.tile([128, T], f32, name=f"yo{j}",
                                    tag=f"yo{j}") for j in range(4)]
                with tc.tile_pool(name="p2s", bufs=2, space="PSUM") as p2s, \
                     tc.tile_pool(name="p2y", bufs=4, space="PSUM") as p2y, \
                     tc.tile_pool(name="p2e", bufs=3) as p2e, \
                     tc.tile_pool(name="p2st", bufs=4) as p2st:
                    for j in range(4):        # head pair (2j, 2j+1)
                        for qc in range(2):   # q chunk of 1024
                            spsA = p2s.tile([128, 1024], f32, name="spsA",
                                            tag="sps")
                            spsB = p2s.tile([128, 1024], f32, name="spsB",
                                            tag="sps")
                            yps = [[p2y.tile([65, 512], f32,
                                             name=f"yps{hh}_{n}", tag="yps")
                                    for n in range(2)] for hh in range(2)]
                            for tt in range(NT):
                                for n in range(2):
                                    qsl = ds(qc * 1024 + n * 512, 512)
                                    nc.tensor.matmul(
                                        spsA[:, ts(n, 512)],
                                        qkt[4 + j][0:64, ts(tt, 128)],
                                        qkt[j][0:64, qsl],
                                        start=True, stop=True)
                                    nc.tensor.matmul(
                                        spsB[:, ts(n, 512)],
                                        qkt[4 + j][64:128, ts(tt, 128)],
                                        qkt[j][64:128, qsl],
                                        start=True, stop=True)
                                expA = p2e.tile([128, 1024], f32, name="expA",
                                                tag="expA")
                                expB = p2e.tile([128, 1024], f32, name="expB",
                                                tag="expB")
                                nc.scalar.activation(expA, spsA, EXP,
                                                     scale=0.125)
                                nc.scalar.activation(expB, spsB, EXP,
                                                     scale=0.125)
                                for n in range(2):
                                    nc.tensor.matmul(
                                        yps[0][n][0:65, :],
                                        vbuf[tt][:, 2 * j, 0:D + 1],
                                        expA[:, ts(n, 512)],
                                        start=(tt == 0), stop=(tt == NT - 1))
                                    nc.tensor.matmul(
                                        yps[1][n][0:65, :],
                                        vbuf[tt][:, 2 * j + 1, 0:D + 1],
                                        expB[:, ts(n, 512)],
                                        start=(tt == 0), stop=(tt == NT - 1))
                            # unload accumulators: y rows + denominator row
                            for hh in range(2):
                                hl = 2 * j + hh
                                for n in range(2):
                                    qs = qc * 1024 + n * 512
                                    yp = yps[hh][n]
                                    stg = p2st.tile([128, 512], f32,
                                                    name="stg", tag="stg")
                                    if hh == 0:
                                        nc.vector.tensor_copy(
                                            youtT[j][0:64, ds(qs, 512)],
                                            yp[0:64, :])
                                        nc.vector.tensor_copy(
                                            stg[64:65, :], yp[64:65, :])
                                    else:
                                        nc.vector.tensor_copy(
                                            stg[0:65, :], yp[0:65, :])
                                        nc.sync.dma_start(
                                            youtT[j][64:128, ds(qs, 512)],
                                            stg[0:64, :])
                                    nc.sync.dma_start(
                                        den_dram[hl:hl + 1, ds(qs, 512)],
                                        stg[64:65, :])

                # ---------- phase 3: normalize + output projection ----------
                with tc.tile_pool(name="p3", bufs=1) as p3, \
                     tc.tile_pool(name="p3bc", bufs=3) as p3bc, \
                     tc.tile_pool(name="p3o", bufs=3) as p3o, \
                     tc.tile_pool(name="p3ps", bufs=2, space="PSUM") as p3ps, \
                     tc.tile_pool(name="p3w", bufs=1) as p3w:
                    wp_sb = [p3w.tile([128, C], f32, name=f"wp{k}",
                                      tag=f"wp{k}") for k in range(4)]
                    for k in range(4):
                        nc.sync.dma_start(wp_sb[k], wpT[ts(k, 128), :])

                    densb = p3.tile([HPG, T], f32, name="densb")
                    nc.sync.dma_start(densb[0:HPG, :], den_dram[:, :])
                    recsb = p3.tile([HPG, T], f32, name="recsb")
                    nc.vector.reciprocal_approx_fast(recsb[0:HPG, :],
                                                     densb[0:HPG, :])
                    nc.sync.dma_start(rec_dram[:, :], recsb[0:HPG, :])

                    for h in range(HPG):
                        j, rb = h // 2, 64 * (h % 2)
                        for q4 in range(4):
                            bc = p3bc.tile([128, 512], f32, name="bc",
                                           tag="bc")
                            src = bass.AP(
                                tensor=rec_dram.tensor,
                                offset=h * T + q4 * 512,
                                ap=[[0, 64], [1, 512]])
                            nc.gpsimd.dma_start(out=bc[rb:rb + 64, :],
                                                in_=src)
                            nc.vector.tensor_mul(
                                youtT[j][rb:rb + 64, ts(q4, 512)],
                                youtT[j][rb:rb + 64, ts(q4, 512)],
                                bc[rb:rb + 64, :])

                    for tm in range(NT):
                        ops = p3ps.tile([128, 1024], f32, name="ops",
                                        tag="ops")
                        for k in range(4):
                            for n in range(2):
                                nc.tensor.matmul(
                                    ops[:, ts(n, 512)],
                                    youtT[k][:, ts(tm, 128)],
                                    wp_sb[k][:, ts(n, 512)],
                                    start=(k == 0), stop=(k == 3))
                        osb = p3o.tile([128, 1024], f32, name="osb",
                                       tag="osb")
                        nc.scalar.copy(osb, ops)
                        nc.sync.dma_start(out[ts(tm, 128), :], osb)

    nc.compile()
    return nc


def _get_nc():
    if "nc" not in _cache:
        _cache["nc"] = _build()
    return _cache["nc"]


def make_in_maps(x, W_attn, W_proj):
    x = np.ascontiguousarray(x, dtype=np.float32)
    W_attn = np.asarray(W_attn, dtype=np.float32)
    W_proj = np.asarray(W_proj, dtype=np.float32)
    in_maps = []
    for core in range(NCORES):
        b, hg = divmod(core, 2)
        lo, hi = hg * CL, (hg + 1) * CL
        wqk = np.concatenate([W_attn[lo:hi], W_attn[C + lo:C + hi]], axis=0)
        in_maps.append({
            "xT": np.ascontiguousarray(x[b].T),
            "wqkT": np.ascontiguousarray(wqk.T),
            "wvT": np.ascontiguousarray(W_attn[2 * C + lo:2 * C + hi].T),
            "wpT": np.ascontiguousarray(W_proj[:, lo:hi].T),
        })
    return in_maps


def combine(results, b_proj):
    out = np.empty((B, T, C), dtype=np.float32)
    for b in range(B):
        out[b] = results[2 * b]["out"] + results[2 * b + 1]["out"]
    out += np.asarray(b_proj, dtype=np.float32)
    return out


def kernel(x, W_attn, W_proj, b_proj):
    from concourse.bass_utils import run_bass_kernel_spmd

    nc = _get_nc()
    in_maps = make_in_maps(x, W_attn, W_proj)
    res = run_bass_kernel_spmd(nc, in_maps, core_ids=list(range(NCORES)))
    return combine(res.results, b_proj)


# revision 6
# speedup vs baseline: 1.1024x; 1.1024x over previous
"""Multi-head attention forward on 8 Trainium2 NeuronCores.

Problem: x[4,2048,1024], W_attn[3072,1024], W_proj[1024,1024], b_proj[1024]
  qkv = x @ W_attn.T ; per-head softmax(q k^T / sqrt(64)) @ v ; out = y @ W_proj.T + b

Sharding: core = (batch b, head-group hg), b = core//2, hg = core%2.
Each core computes its batch's attention output for its 8 heads plus the
partial output projection over its 512 y-channels; the host sums the two
partials per batch and adds the bias.

On-core layout (all fp32):
  - The contraction of every matmul must sit on the SBUF partition dim, so the
    host ships x and the weights pre-transposed: xT[c,t], wqkT[c,o], wvT[c,o],
    wpT[c_local,o].
  - q,k are produced transposed (qkT[o,t]); attention scores are computed as
    s^T[k,q] = (kT)^T-style matmuls with K=64, two heads packed into the
    128-row PE array (rows 0:64 / 64:128).
  - softmax runs without max-subtraction (inputs are ~N(0,1) after scaling so
    exp never overflows); exp is fused with the 1/8 scale on the scalar engine.
  - v carries an extra all-ones column per head, so the p@v matmul's 65th
    output row is the softmax denominator for free.
  - y^T is normalized via a DRAM round-trip of the 16K denominators
    (reciprocal on DVE, then partition-broadcast DMA loads) and fed straight
    into the output projection as the stationary operand.
"""

import sys

import numpy as np

if "/opt/trn_rl_repo" not in sys.path:
    sys.path.insert(0, "/opt/trn_rl_repo")

B, T, C, H, D = 4, 2048, 1024, 16, 64
HPG = H // 2          # heads per core group = 8
CL = HPG * D          # local y-channels = 512
KC = C // 128         # 8 contraction tiles over c
NT = T // 128         # 16 tiles over t
NCORES = 8

_cache = {}


def _build():
    import concourse.bacc as bacc
    import concourse.bass as bass
    import concourse.mybir as mybir
    import concourse.tile as tile
    from concourse.bass import ds, ts

    f32 = mybir.dt.float32
    f32r = mybir.dt.float32r
    EXP = mybir.ActivationFunctionType.Exp

    nc = bacc.Bacc("TRN2", target_bir_lowering=False, debug=False,
                   enable_asserts=False)

    xT = nc.dram_tensor("xT", [C, T], f32r, kind="ExternalInput").ap()
    wqkT = nc.dram_tensor("wqkT", [C, 2 * CL], f32r, kind="ExternalInput").ap()
    wvT = nc.dram_tensor("wvT", [C, CL], f32r, kind="ExternalInput").ap()
    wpT = nc.dram_tensor("wpT", [CL, C], f32r, kind="ExternalInput").ap()
    out = nc.dram_tensor("out", [T, C], f32, kind="ExternalOutput").ap()
    den_dram = nc.dram_tensor("den_scr", [HPG, T], f32, kind="Internal").ap()
    rec_dram = nc.dram_tensor("rec_scr", [HPG, T], f32, kind="Internal").ap()

    with tile.TileContext(nc) as tc:
        with tc.tile_pool(name="pers", bufs=1) as pers:
            # persistent: q/k transposed [o,t] (tiles 0-3 q, 4-7 k; head pair
            # 2m/2m+1 in rows 0:64/64:128) and v in [t, head, d+ones] layout
            qkt = [pers.tile([128, T], f32r, name=f"qkt{m}", tag=f"qkt{m}")
                   for m in range(8)]
            vbuf = [pers.tile([128, HPG, D + 1], f32r, name=f"vb{t}",
                              tag=f"vb{t}") for t in range(NT)]
            ones8 = pers.tile([128, HPG], f32, name="ones8")
            nc.vector.memset(ones8, 1.0)

            # ---------- phase 1: qkv projection ----------
            with tc.tile_pool(name="p1w", bufs=1) as p1w, \
                 tc.tile_pool(name="p1x", bufs=2) as p1x, \
                 tc.tile_pool(name="p1qk", bufs=2, space="PSUM") as p1qk, \
                 tc.tile_pool(name="p1v", bufs=2, space="PSUM") as p1v:
                wqk_sb = [p1w.tile([128, 2 * CL], f32r, name=f"wqk{k}",
                                   tag=f"wqk{k}") for k in range(KC)]
                wv_sb = [p1w.tile([128, CL], f32r, name=f"wv{k}",
                                  tag=f"wv{k}") for k in range(KC)]
                for k in range(KC):
                    nc.sync.dma_start(wqk_sb[k], wqkT[ts(k, 128), :])
                    nc.sync.dma_start(wv_sb[k], wvT[ts(k, 128), :])

                for half in range(2):
                    xq = {}
                    for k in range(KC):
                        for nq in range(2):
                            qq = 2 * half + nq
                            xt = p1x.tile([128, 512], f32r, name=f"xq{k}_{qq}",
                                          tag=f"xq{k}")
                            nc.sync.dma_start(
                                xt, xT[ts(k, 128), ts(qq, 512)])
                            xq[(k, qq)] = xt
                    # qk projection: qkt[m][o, t_half] += wqk^T x
                    for m in range(8):
                        qps = p1qk.tile([128, 1024], f32, name="qps",
                                        tag="qps")
                        for k in range(KC):
                            for nq in range(2):
                                nc.tensor.matmul(
                                    qps[:, ts(nq, 512)],
                                    wqk_sb[k][:, ts(m, 128)],
                                    xq[(k, 2 * half + nq)],
                                    start=(k == 0), stop=(k == KC - 1))
                        nc.scalar.copy(qkt[m][:, ds(half * 1024, 1024)], qps)
                    # v projection into [t, head, d] with ones column
                    for tl in range(8):
                        tt = half * 8 + tl
                        vps = p1v.tile([128, 512], f32, name="vps", tag="vps")
                        for k in range(KC):
                            nc.tensor.matmul(
                                vps,
                                xq[(k, 2 * half + tl // 4)][:, ds((tl % 4) * 128, 128)],
                                wv_sb[k],
                                start=(k == 0), stop=(k == KC - 1))
                        nc.vector.tensor_copy(vbuf[tt][:, :, D:D + 1], ones8)
                        nc.vector.tensor_copy(
                            vbuf[tt][:, :, 0:D],
                            vps.rearrange("p (h d) -> p h d", d=D))

            # ---------- phase 2: attention ----------
            with tc.tile_pool(name="yout", bufs=1) as youtp:
                youtT = [youtp.tile([128, T], f32r, name=f"yo{j}",
                                    tag=f"yo{j}") for j in range(4)]
                with tc.tile_pool(name="p2s", bufs=2, space="PSUM") as p2s, \
                     tc.tile_pool(name="p2y", bufs=4, space="PSUM") as p2y, \
                     tc.tile_pool(name="p2e", bufs=3) as p2e, \
                     tc.tile_pool(name="p2den", bufs=1) as p2den, \
                     tc.tile_pool(name="p2bc", bufs=3) as p2bc, \
                     tc.tile_pool(name="p2st", bufs=4) as p2st:
                    for j in range(4):        # head pair (2j, 2j+1)
                        for qc in range(2):   # q chunk of 1024
                            spsA = p2s.tile([128, 1024], f32, name="spsA",
                                            tag="sps")
                            spsB = p2s.tile([128, 1024], f32, name="spsB",
                                            tag="sps")
                            yps = [[p2y.tile([65, 512], f32,
                                             name=f"yps{hh}_{n}", tag="yps")
                                    for n in range(2)] for hh in range(2)]
                            for tt in range(NT):
                                for n in range(2):
                                    qsl = ds(qc * 1024 + n * 512, 512)
                                    nc.tensor.matmul(
                                        spsA[:, ts(n, 512)],
                                        qkt[4 + j][0:64, ts(tt, 128)],
                                        qkt[j][0:64, qsl],
                                        start=True, stop=True)
                                    nc.tensor.matmul(
                                        spsB[:, ts(n, 512)],
                                        qkt[4 + j][64:128, ts(tt, 128)],
                                        qkt[j][64:128, qsl],
                                        start=True, stop=True)
                                expA = p2e.tile([128, 1024], f32r, name="expA",
                                                tag="expA")
                                expB = p2e.tile([128, 1024], f32r, name="expB",
                                                tag="expB")
                                nc.scalar.activation(expA, spsA, EXP,
                                                     scale=0.125)
                                nc.scalar.activation(expB, spsB, EXP,
                                                     scale=0.125)
                                for n in range(2):
                                    nc.tensor.matmul(
                                        yps[0][n][0:65, :],
                                        vbuf[tt][:, 2 * j, 0:D + 1],
                                        expA[:, ts(n, 512)],
                                        start=(tt == 0), stop=(tt == NT - 1))
                                    nc.tensor.matmul(
                                        yps[1][n][0:65, :],
                                        vbuf[tt][:, 2 * j + 1, 0:D + 1],
                                        expB[:, ts(n, 512)],
                                        start=(tt == 0), stop=(tt == NT - 1))
                            # unload accumulators: y rows + denominator row
                            for hh in range(2):
                                hl = 2 * j + hh
                                for n in range(2):
                                    qs = qc * 1024 + n * 512
                                    yp = yps[hh][n]
                                    stg = p2st.tile([128, 512], f32,
                                                    name="stg", tag="stg")
                                    if hh == 0:
                                        nc.vector.tensor_copy(
                                            youtT[j][0:64, ds(qs, 512)],
                                            yp[0:64, :])
                                    else:
                                        stgy = p2st.tile([128, 512], f32r,
                                                         name="stgy",
                                                         tag="stgy")
                                        nc.vector.tensor_copy(
                                            stgy[0:64, :], yp[0:64, :])
                                        nc.sync.dma_start(
                                            youtT[j][64:128, ds(qs, 512)],
                                            stgy[0:64, :])
                                    nc.vector.tensor_copy(
                                        stg[64:65, :], yp[64:65, :])
                                    nc.sync.dma_start(
                                        den_dram[hl:hl + 1, ds(qs, 512)],
                                        stg[64:65, :])
                        # normalize this pair's y^T while later pairs compute
                        densb = p2den.tile([2, T], f32, name="densb",
                                           tag="densb", bufs=2)
                        recsb = p2den.tile([2, T], f32, name="recsb",
                                           tag="recsb", bufs=2)
                        nc.sync.dma_start(densb[0:2, :],
                                          den_dram[2 * j:2 * j + 2, :])
                        nc.vector.reciprocal_approx_fast(
                            recsb[0:2, :], densb[0:2, :])
                        nc.sync.dma_start(rec_dram[2 * j:2 * j + 2, :],
                                          recsb[0:2, :])
                        for hh in range(2):
                            h = 2 * j + hh
                            rb = 64 * hh
                            for q4 in range(4):
                                bc = p2bc.tile([128, 512], f32, name="bc",
                                               tag="bc")
                                src = bass.AP(
                                    tensor=rec_dram.tensor,
                                    offset=h * T + q4 * 512,
                                    ap=[[0, 64], [1, 512]])
                                nc.gpsimd.dma_start(out=bc[rb:rb + 64, :],
                                                    in_=src)
                                nc.vector.tensor_mul(
                                    youtT[j][rb:rb + 64, ts(q4, 512)],
                                    youtT[j][rb:rb + 64, ts(q4, 512)],
                                    bc[rb:rb + 64, :])

                # ---------- phase 3: output projection ----------
                with tc.tile_pool(name="p3o", bufs=3) as p3o, \
                     tc.tile_pool(name="p3ps", bufs=2, space="PSUM") as p3ps, \
                     tc.tile_pool(name="p3w", bufs=1) as p3w:
                    wp_sb = [p3w.tile([128, C], f32r, name=f"wp{k}",
                                      tag=f"wp{k}") for k in range(4)]
                    for k in range(4):
                        nc.sync.dma_start(wp_sb[k], wpT[ts(k, 128), :])

                    for tm in range(NT):
                        ops = p3ps.tile([128, 1024], f32, name="ops",
                                        tag="ops")
                        for k in range(4):
                            for n in range(2):
                                nc.tensor.matmul(
                                    ops[:, ts(n, 512)],
                                    youtT[k][:, ts(tm, 128)],
                                    wp_sb[k][:, ts(n, 512)],
                                    start=(k == 0), stop=(k == 3))
                        osb = p3o.tile([128, 1024], f32, name="osb",
                                       tag="osb")
                        nc.scalar.copy(osb, ops)
                        nc.sync.dma_start(out[ts(tm, 128), :], osb)

    nc.compile()
    return nc


def _get_nc():
    if "nc" not in _cache:
        _cache["nc"] = _build()
    return _cache["nc"]


def make_in_maps(x, W_attn, W_proj):
    x = np.ascontiguousarray(x, dtype=np.float32)
    W_attn = np.asarray(W_attn, dtype=np.float32)
    W_proj = np.asarray(W_proj, dtype=np.float32)
    in_maps = []
    for core in range(NCORES):
        b, hg = divmod(core, 2)
        lo, hi = hg * CL, (hg + 1) * CL
        wqk = np.concatenate([W_attn[lo:hi], W_attn[C + lo:C + hi]], axis=0)
        in_maps.append({
            "xT": np.ascontiguousarray(x[b].T),
            "wqkT": np.ascontiguousarray(wqk.T),
            "wvT": np.ascontiguousarray(W_attn[2 * C + lo:2 * C + hi].T),
            "wpT": np.ascontiguousarray(W_proj[:, lo:hi].T),
        })
    return in_maps


def combine(results, b_proj):
    out = np.empty((B, T, C), dtype=np.float32)
    for b in range(B):
        out[b] = results[2 * b]["out"] + results[2 * b + 1]["out"]
    out += np.asarray(b_proj, dtype=np.float32)
    return out


def kernel(x, W_attn, W_proj, b_proj):
    from concourse.bass_utils import run_bass_kernel_spmd

    nc = _get_nc()
    in_maps = make_in_maps(x, W_attn, W_proj)
    res = run_bass_kernel_spmd(nc, in_maps, core_ids=list(range(NCORES)))
    return combine(res.results, b_proj)
